# revision 9
# baseline (speedup 1.0000x reference)
"""Bidirectional 3-layer GRU (T=512, B=64, I=H=512) on 8 trn2 NeuronCores.

Strategy:
- Direction-split: cores 0-3 run the forward scans, cores 4-7 the backward
  scans (a backward GRU over x == forward GRU over time-reversed x). Batch is
  quartered across the 4 cores of each group (BL=16 per core).
- Everything on-device runs in "transposed" layout [feature -> partitions,
  (t, batch) -> free] so the recurrent matmul is weight-stationary
  (lhsT = W.T tiles) and the gate elementwise ops use all 128 lanes.
- Per layer: batched input projection gih = W_ih @ x.T (bf16 matmuls,
  fp32 psum), then the sequential 512-step scan (48 small matmuls per step,
  LDW-bound), h state kept fp32, cast to bf16 for the matmul rhs.
- Layer boundaries need the *other* direction's outputs: pairwise
  AllReduce(add) of fp32 z-buffers over {i, i+4}, then peer = sum - own,
  time-reversed into the core's own scan order. Host-side weight-column
  permutation absorbs the [own; peer] feature order, so one SPMD program
  works for both directions.
"""

import numpy as np

import concourse.bass as bass
import concourse.bacc as bacc
import concourse.mybir as mybir
import concourse.tile as tile
from concourse import bass_utils

T, B, I, H, L = 512, 64, 512, 512, 3
NCORES = 8
GD = 4            # cores per direction group
BL = B // GD      # 16 batch per core
HC = H // 128     # 4 feature chunks of 128
MC = 3 * H // 128  # 12 output-row chunks of 3H

fp32 = mybir.dt.float32
bf16 = mybir.dt.bfloat16
AF = mybir.ActivationFunctionType
ALU = mybir.AluOpType

REPLICA_PAIRS = [[0, 4], [1, 5], [2, 6], [3, 7]]


def build_program(ts=T, debug_taps=False):
    """Build + compile the SPMD program for ts timesteps. Returns nc."""
    tb = ts * BL
    nj = tb // 512  # projection column chunks

    nc = bacc.Bacc("TRN2", target_bir_lowering=False, debug=False,
                   num_devices=NCORES)
    gdbg = zdbg = None
    if debug_taps:
        gdbg = nc.dram_tensor("gdbg", [ts, 128, HC, 3, BL], fp32,
                              kind="ExternalOutput").ap()
        zdbg = nc.dram_tensor("zdbg", [HC, 128, ts, BL], fp32,
                              kind="ExternalOutput").ap()

    xT = nc.dram_tensor("xT", [HC, 128, tb], bf16, kind="ExternalInput").ap()
    wih0 = nc.dram_tensor("wih0", [HC, 128, MC, 128], bf16,
                          kind="ExternalInput").ap()
    wih12 = nc.dram_tensor("wih12", [2, 2 * HC, 128, MC, 128], bf16,
                           kind="ExternalInput").ap()
    whh = nc.dram_tensor("whh", [L, HC, 128, MC, 128], bf16,
                         kind="ExternalInput").ap()
    brzn = nc.dram_tensor("brzn", [L, MC, 128], fp32, kind="ExternalInput").ap()
    bhhn = nc.dram_tensor("bhhn", [L, HC, 128], fp32, kind="ExternalInput").ap()
    h0T = nc.dram_tensor("h0T", [L, HC, 128, BL], fp32,
                         kind="ExternalInput").ap()
    ident = nc.dram_tensor("ident", [128, 128], fp32, kind="ExternalInput").ap()
    yout = nc.dram_tensor("y", [ts, BL, H], fp32, kind="ExternalOutput").ap()

    with tile.TileContext(nc) as tc:
        with tc.tile_pool(name="dram", bufs=1, space="DRAM") as dram:
            gbuf = dram.tile([ts, 128, HC, 3, BL], fp32)
            zf32 = dram.tile([HC, 128, ts, BL], fp32)
            zbf = dram.tile([HC, 128, ts, BL], bf16)
            ccsum = dram.tile([HC, 128, ts, BL], fp32)
            xpeer = dram.tile([HC, 128, ts, BL], bf16)
            y3T = dram.tile([HC, 128, ts, BL], fp32)

            with tc.tile_pool(name="const", bufs=1) as constp:
                id_sb = constp.tile([128, 128], fp32)
                nc.sync.dma_start(id_sb[:], ident[:])

                for l in range(L):
                    kc = HC if l == 0 else 2 * HC

                    # ---------------- input projection ----------------
                    with (
                        tc.tile_pool(name="wproj", bufs=1) as wp,
                        tc.tile_pool(name="rhs", bufs=2) as rp,
                        tc.tile_pool(name="evac", bufs=4) as ep,
                        tc.tile_pool(name="ppsum", bufs=4, space="PSUM") as pp,
                    ):
                        wsb = wp.tile([128, kc * MC * 128], bf16)
                        wsrc = wih0 if l == 0 else wih12[l - 1]
                        nc.sync.dma_start(
                            wsb[:].rearrange("p (k m n) -> p k m n",
                                             k=kc, m=MC),
                            wsrc.rearrange("k p m n -> p k m n"))
                        bsb = wp.tile([128, MC], fp32)
                        nc.sync.dma_start(bsb[:], brzn[l].rearrange("m p -> p m"))

                        for j in range(nj):
                            rts = []
                            for k in range(kc):
                                rt = rp.tile([128, 512], bf16, tag=f"rhs{k}")
                                if l == 0:
                                    src = xT[k][:, j * 512:(j + 1) * 512]
                                elif k < HC:
                                    src = zbf[k].rearrange("p t b -> p (t b)")[
                                        :, j * 512:(j + 1) * 512]
                                else:
                                    src = xpeer[k - HC].rearrange(
                                        "p t b -> p (t b)")[
                                        :, j * 512:(j + 1) * 512]
                                nc.sync.dma_start(rt[:], src)
                                rts.append(rt)
                            for m in range(MC):
                                ps = pp.tile([128, 512], fp32)
                                for k in range(kc):
                                    nc.tensor.matmul(
                                        ps[:],
                                        wsb[:, (k * MC + m) * 128:
                                            (k * MC + m + 1) * 128],
                                        rts[k][:],
                                        start=(k == 0), stop=(k == kc - 1))
                                ev = ep.tile([128, 512], fp32)
                                nc.scalar.activation(
                                    ev[:], ps[:], AF.Identity,
                                    bias=bsb[:, m:m + 1])
                                c_, g_ = m % HC, m // HC
                                nc.sync.dma_start(
                                    gbuf[j * 32:(j + 1) * 32, :, c_, g_, :]
                                    .rearrange("t p b -> p t b"),
                                    ev[:].rearrange("p (t b) -> p t b", b=BL))

                    if debug_taps and l == 0:
                        nc.sync.dma_start(gdbg[:], gbuf[:])

                    # ---------------- recurrent scan ----------------
                    with (
                        tc.tile_pool(name="wscan", bufs=1) as wsp,
                        tc.tile_pool(name="state", bufs=1) as stp,
                        tc.tile_pool(name="gt", bufs=3) as gp,
                        tc.tile_pool(name="ew", bufs=3) as ewp,
                        tc.tile_pool(name="spsum", bufs=2, space="PSUM") as sp,
                    ):
                        wsb2 = wsp.tile([128, HC * MC * 128], bf16)
                        nc.sync.dma_start(
                            wsb2[:].rearrange("p (k m n) -> p k m n",
                                              k=HC, m=MC),
                            whh[l].rearrange("k p m n -> p k m n"))
                        bnsb = wsp.tile([128, HC], fp32)
                        nc.sync.dma_start(
                            bnsb[:], bhhn[l].rearrange("c p -> p c"))

                        hT, hB = [], []
                        for c in range(HC):
                            ht = stp.tile([128, BL], fp32, tag=f"hT{c}")
                            hb = stp.tile([128, BL], bf16, tag=f"hB{c}")
                            nc.sync.dma_start(ht[:], h0T[l, c])
                            nc.scalar.activation(hb[:], ht[:], AF.Copy)
                            hT.append(ht)
                            hB.append(hb)

                        for t in range(ts):
                            gt = gp.tile([128, HC, 3, BL], fp32)
                            nc.sync.dma_start(gt[:], gbuf[t])
                            pss = []
                            for c in range(HC):
                                ps_c = sp.tile([128, 3, BL], fp32,
                                               tag=f"ps{c}", name=f"ps{c}")
                                pss.append(ps_c)
                            for k in range(HC):
                                for c in range(HC):
                                    for g in range(3):
                                        m = g * HC + c
                                        # one bank holds all 3 gate regions:
                                        # start (bank-wide has_written clear)
                                        # only on the first MM into the bank
                                        nc.tensor.matmul(
                                            pss[c][:, g, :],
                                            wsb2[:, (k * MC + m) * 128:
                                                 (k * MC + m + 1) * 128],
                                            hB[k][:],
                                            start=(k == 0 and g == 0),
                                            stop=(k == HC - 1 and g == 2),
                                            skip_group_check=True)
                            for c in range(HC):
                                az = ewp.tile([128, 2, BL], fp32, tag=f"az{c}")
                                nc.vector.tensor_add(
                                    az[:], pss[c][:, 0:2, :], gt[:, c, 0:2, :])
                                rz = ewp.tile([128, 2, BL], fp32, tag=f"rz{c}")
                                nc.scalar.activation(rz[:], az[:], AF.Sigmoid)
                                tn = ewp.tile([128, BL], fp32, tag=f"tn{c}")
                                nc.vector.scalar_tensor_tensor(
                                    tn[:], pss[c][:, 2, :], bnsb[:, c:c + 1],
                                    rz[:, 0, :], op0=ALU.add, op1=ALU.mult)
                                an = ewp.tile([128, BL], fp32, tag=f"an{c}")
                                nc.vector.tensor_add(an[:], tn[:], gt[:, c, 2, :])
                                nn = ewp.tile([128, BL], fp32, tag=f"nn{c}")
                                nc.scalar.activation(nn[:], an[:], AF.Tanh)
                                d = ewp.tile([128, BL], fp32, tag=f"d{c}")
                                nc.vector.tensor_sub(d[:], hT[c][:], nn[:])
                                zd = ewp.tile([128, BL], fp32, tag=f"zd{c}")
                                nc.vector.tensor_mul(zd[:], rz[:, 1, :], d[:])
                                nc.vector.tensor_add(hT[c][:], nn[:], zd[:])
                                nc.scalar.activation(hB[c][:], hT[c][:], AF.Copy)
                                if l < L - 1:
                                    nc.sync.dma_start(zf32[c][:, t, :], hT[c][:])
                                else:
                                    nc.sync.dma_start(y3T[c][:, t, :], hT[c][:])

                    # ---------------- boundary exchange ----------------
                    if l < L - 1:
                        nc.gpsimd.collective_compute(
                            "AllReduce", ALU.add,
                            replica_groups=REPLICA_PAIRS,
                            ins=[zf32.opt()], outs=[ccsum.opt()])
                        with tc.tile_pool(name="xch", bufs=4) as xp:
                            for c in range(HC):
                                for jt in range(ts // 32):
                                    a = xp.tile([128, 512], fp32, tag="a")
                                    bq = xp.tile([128, 512], fp32, tag="b")
                                    nc.sync.dma_start(
                                        a[:],
                                        ccsum[c][:, jt * 32:(jt + 1) * 32, :]
                                        .rearrange("p t b -> p (t b)"))
                                    nc.sync.dma_start(
                                        bq[:],
                                        zf32[c][:, jt * 32:(jt + 1) * 32, :]
                                        .rearrange("p t b -> p (t b)"))
                                    o = xp.tile([128, 512], bf16, tag="o")
                                    nc.vector.tensor_sub(o[:], a[:], bq[:])
                                    zc = xp.tile([128, 512], bf16, tag="z")
                                    nc.vector.tensor_copy(zc[:], bq[:])
                                    rev = xpeer[c][:, ts - (jt + 1) * 32:
                                                   ts - jt * 32, :][:, ::-1, :]
                                    nc.sync.dma_start(
                                        rev,
                                        o[:].rearrange("p (t b) -> p t b", b=BL))
                                    nc.sync.dma_start(
                                        zbf[c][:, jt * 32:(jt + 1) * 32, :]
                                        .rearrange("p t b -> p (t b)"), zc[:])

                if debug_taps:
                    nc.sync.dma_start(zdbg[:], y3T[:])

                # ---------------- final un-transpose ----------------
                with (
                    tc.tile_pool(name="tp", bufs=4) as tpo,
                    tc.tile_pool(name="tpp", bufs=4, space="PSUM") as tpp,
                ):
                    for c in range(HC):
                        for j in range(tb // 128):
                            it = tpo.tile([128, 128], fp32, tag="in")
                            nc.sync.dma_start(
                                it[:],
                                y3T[c].rearrange("p t b -> p (t b)")[
                                    :, j * 128:(j + 1) * 128])
                            pt = tpp.tile([128, 128], fp32)
                            nc.tensor.transpose(pt[:], it[:], id_sb[:])
                            ot = tpo.tile([128, 128], fp32, tag="out")
                            nc.vector.tensor_copy(ot[:], pt[:])
                            nc.sync.dma_start(
                                yout.rearrange("t b h -> (t b) h")[
                                    j * 128:(j + 1) * 128,
                                    c * 128:(c + 1) * 128], ot[:])

    nc.compile()
    return nc


def make_in_maps(x, h0, weights, ts=T):
    """Host-side sharding/layout prep. Returns per-core input dicts."""
    x = np.asarray(x, np.float32)
    h0 = np.asarray(h0, np.float32)
    weights = [np.asarray(w, np.float32) for w in weights]
    nbf = mybir.dt.np(bf16)

    ident = np.eye(128, dtype=np.float32)
    in_maps = []
    for core in range(NCORES):
        d, q = core // GD, core % GD
        xc = x[:ts, q * BL:(q + 1) * BL, :]
        if d == 1:
            xc = xc[::-1]
        xTa = np.ascontiguousarray(xc.transpose(2, 0, 1)).reshape(
            HC, 128, ts * BL).astype(nbf)

        wih0a = None
        wih12a = np.empty((2, 2 * HC, 128, MC, 128), nbf)
        whha = np.empty((L, HC, 128, MC, 128), nbf)
        brzna = np.empty((L, MC, 128), np.float32)
        bhhna = np.empty((L, HC, 128), np.float32)
        h0Ta = np.empty((L, HC, 128, BL), np.float32)
        for l in range(L):
            wi, wh, bi, bh = weights[l * 8 + d * 4: l * 8 + d * 4 + 4]
            wiT = np.ascontiguousarray(wi.T)  # [in, 3H]
            if l == 0:
                wih0a = wiT.reshape(HC, 128, MC, 128).astype(nbf)
            else:
                if d == 1:
                    wiT = np.concatenate([wiT[H:], wiT[:H]], axis=0)
                wih12a[l - 1] = wiT.reshape(2 * HC, 128, MC, 128).astype(nbf)
            whha[l] = np.ascontiguousarray(wh.T).reshape(
                HC, 128, MC, 128).astype(nbf)
            bfull = bi.copy()
            bfull[:2 * H] += bh[:2 * H]
            brzna[l] = bfull.reshape(MC, 128)
            bhhna[l] = bh[2 * H:].reshape(HC, 128)
            h0Ta[l] = np.ascontiguousarray(
                h0[2 * l + d, q * BL:(q + 1) * BL, :].T).reshape(HC, 128, BL)

        in_maps.append({
            "xT": xTa, "wih0": wih0a, "wih12": wih12a, "whh": whha,
            "brzn": brzna, "bhhn": bhhna, "h0T": h0Ta, "ident": ident,
        })
    return in_maps


def assemble_output(results, ts=T):
    y = np.empty((ts, B, 2 * H), np.float32)
    for q in range(GD):
        y[:, q * BL:(q + 1) * BL, :H] = results[q]["y"]
        y[:, q * BL:(q + 1) * BL, H:] = results[GD + q]["y"][::-1]
    return y


_NC_CACHE = {}


def kernel(x, h0, weights):
    if T not in _NC_CACHE:
        _NC_CACHE[T] = build_program(T)
    nc = _NC_CACHE[T]
    in_maps = make_in_maps(x, h0, weights, T)
    res = bass_utils.run_bass_kernel_spmd(
        nc, in_maps, core_ids=list(range(NCORES)))
    return assemble_output(res.results, T)
